# revision 15
# baseline (speedup 1.0000x reference)
"""Trainium2 Bass kernel for the custom MHA problem nn_CustomMHA_14551349198906.

Computation (per batch b):
    t = x @ w_qkv.T ; q,k,v = heads of t        # (S, 3D), H=16 heads of 64
    scores = einsum('sid,sjd->sij', q, k)/sqrt(D)   # per-token 16x16
    lower-tri mask, exact-0 -> -inf, softmax over j
    y' = p @ v ; out = y' @ w_o.T

Distribution: pure data-parallel over the batch (B=8 -> one NeuronCore per
batch element); no collectives.

v2 design (per 128-token tile, tokens on partitions; CoreSim 729us
DVE-bound vs 1302us for the bf16x3/all-DVE baseline, ~735us expected
on real HW with measured Pool rates; HW rel err 1.473e-2 < 2e-2 gate):
  - PE: all of t = x@w_qkv.T in single-pass f32r (1 cycle/row at
    512-wide output, same rate as bf16, vs 3 passes for the bf16 hi/lo
    split -- costs ~1.3e-2 of softmax-logit error but halves PE time),
    y' transpose (fp16 identity matmul), out = y'@w_o.T in bf16.
  - scores (per-token 16x16 Gram, both operands per-token -> PE cannot
    help): per diagonal (i-j): Pool f32 multiply, then for diagonals
    0-3 a Pool fold (add d-halves), then a DVE segmented reduce.
    Segmented (free-axis) reduces are DVE-only (gpsimd tensor_reduce is
    partition-axis only). Real-HW A/B (30k-inst NEFFs, this session)
    puts Q7/Pool TensorTensor at ~2ns/elem -- the 0.42 Multiply
    efficiency, ~1.9x slower than DVE f32 -- so Pool carries only the
    muls + folds for diagonals 0-3/12 -- tuned so Pool's REAL-rate
    time (~22.0us/tile) sits just under the DVE critical path (~22.0);
    the sim prefers more Pool folds but misprices them.
    (The local compiled CoreSim charges Pool 0.83ns/elem; do not trust
    it for Pool-heavy balances.)
  - softmax on DVE (max/sub/sum/recip/normalize, shifted+p in fp16)
    with exp on Act; the score rect is a persistent pair whose -3e38
    pads are memset once (reduces only ever write the valid triangle).
  - p@v in fp16 on DVE: per-pow2-class batched multiplies (2x DVE mode:
    j innermost, stride 1, on all operands) + fold-halves add tree
    (in0/in1 are the two contiguous j-halves -> stays 2x until the last
    level); the last level writes y' rows directly; heads 8-11 use a
    12-wide product with an 8+4 subtree merge instead of padding to 16.
    TensorReduce has NO 2x mode, so the fold tree is ~2x cheaper than
    mul+reduce.
  - startup: a dma_start occupies its issuing engine's queue for the
    whole transfer, so queue assignment is load-ordering: x tiles 0-1
    first on SP, wq0 on Pool (before its memsets would block), wq1/wq4
    on Act (ahead of the first drains), wq2/wq3/wq5/wo filling SP's
    idle window; q-hi/k-lo drain first and diagonals 8-13 run first so
    score work starts one PE-chunk earlier.
  - transposes/wo accumulate in yp-slice completion order (big head
    classes finish first), so the PE output chain overlaps the p@v
    tail; Act replicates 1/Z so the normalize-mul runs at 2x.

The local walrus build encodes at most ONE inline sync-wait per TPB
instruction; split_excess_waits() hoists extra waits onto same-engine NOPs.

v3: the emission loop is software-pipelined (cfg "lag"): each tile's
y'-transpose + out-projection is emitted LAG tiles late, so the in-order
PE queue always has the next tile's t-matmuls ahead of the
wait-for-p@v transpose (PE never idles >3.4us -> HAM stays warm).
cfg "reorder" additionally emits softmax+p@v of tile n-1 before the
score reduces of tile n (3-stage pipe) so Pool's score muls overlap
DVE's p@v; interleaved HW A/B could not separate the two (noise
+-130us/round), and the 2-stage form measured the best median, so
reorder defaults to False.

Accuracy note (empirical, err_exp.py): logits are sigma~256 and softmax is
near-argmax; fp16 anywhere in the q/k matmul or score product/reduce path
blows rel err to 0.03..0.25 vs the 2e-2 gate. f32r matmul + f32 scores is
mandatory; fp16 is only safe downstream of softmax (exp/p/y'/wo paths).
"""

import math
from contextlib import ExitStack

import numpy as np

import bass_rust
import concourse.bass as bass
import concourse.mybir as mybir
import concourse.tile as tile
from concourse.masks import make_identity

F32 = mybir.dt.float32
F16 = mybir.dt.float16
BF16 = mybir.dt.bfloat16
F32R = mybir.dt.float32r

B, S, D, H, DH = 8, 4096, 1024, 16, 64
TILE = 128
KB = D // 128
N_TILES = S // TILE
NEG = -3.0e38
N_CORES = 8
W = 18   # padded softmax row width (W+1 = 19 diagonal stride)

CFG = {
    # diagonals whose score-mul runs on Pool (segmented reduces are
    # DVE-only: gpsimd tensor_reduce supports partition-axis only).
    # 4, 14, 15 run on DVE: real-HW Pool is ~1.9x/elem slower, and the
    # tiny diagonals are overhead-dominated on the Q7.
    "pool_mul_diags": tuple(o for o in range(16) if o not in (4, 14, 15)),
    # softmax sub / normalize-mul engine: "pool" or "dve"
    "sub_eng": "dve",
    "norm_eng": "dve",
    # diagonals whose products get a Pool fold before the DVE reduce
    "l1_diags": (0, 1, 2, 3, 12),
    # (nj, m) p@v fold levels to run on Pool (fp16 on Q7: avoid)
    "pool_pv_folds": (),
    "bufs": 2,
    # tiles of lag between a tile's attention math and its transpose/out-proj
    "lag": 2,
    # emit softmax+p@v of tile n-1 before scores of tile n (3-stage pipe).
    # A/B-interleaved HW timing couldn't separate this from the 2-stage
    # pipe (noise +-130us); the 2-stage form measured the best median.
    "reorder": False,
}

# p@v pow2 classes: (i_start, i_count, padded nj); i=8..11 is handled
# by a dedicated 12-wide 8+4-subtree block sharing these buffer tags.
PV_CLASSES = [(12, 4, 16), (4, 4, 8), (2, 2, 4), (1, 1, 2),
              (0, 1, 1)]


# --------------------------------------------------------------------------
# walrus workaround: hoist excess sync waits onto same-engine NOPs
# --------------------------------------------------------------------------
def split_excess_waits(nc, max_waits=1):
    n_split = 0
    for fn in nc.m.functions:
        for bb in fn.blocks:
            out = []
            changed = False
            for inst in bb.instructions:
                si = inst.sync_info
                waits = list(si.on_wait) if si is not None and si.on_wait else []
                if len(waits) > max_waits:
                    reg = [w for w in waits if getattr(w, "wait_reg", None) is not None]
                    imm = [w for w in waits if getattr(w, "wait_reg", None) is None]
                    kept = reg[:]
                    hoist = []
                    for w in imm:
                        if len(kept) < max_waits:
                            kept.append(w)
                        else:
                            hoist.append(w)
                    if len(kept) > max_waits:
                        raise RuntimeError(
                            f"{inst.name}: {len(reg)} register waits exceed limit")
                    for j, w in enumerate(hoist):
                        nop = mybir.InstNoOp(
                            name=f"{inst.name}-wsplit{j}",
                            ins=[], outs=[], engine=inst.engine)
                        nop.sync_info = bass_rust.SyncInfo(
                            on_wait=[w], on_update=[])
                        out.append(nop)
                        n_split += 1
                    inst.sync_info = bass_rust.SyncInfo(
                        on_wait=kept,
                        on_update=list(si.on_update) if si.on_update else [])
                    changed = True
                out.append(inst)
            if changed:
                bb.instructions = out
    return n_split


# --------------------------------------------------------------------------
# device program
# --------------------------------------------------------------------------
def _rearr_kb(ap):
    return ap.rearrange("(kb p) n -> p kb n", p=128)


def build(nc, cfg):
    bufs = cfg["bufs"]
    pool_mul = set(cfg["pool_mul_diags"])
    l1_diags = set(cfg["l1_diags"])
    sub_eng = cfg["sub_eng"]
    norm_eng = cfg["norm_eng"]

    xT = nc.dram_tensor("xT", (D, S), F32R, kind="ExternalInput").ap()
    wq = nc.dram_tensor("wq", (D, 3 * D), F32R, kind="ExternalInput").ap()
    wo = nc.dram_tensor("wo", (D, D), BF16, kind="ExternalInput").ap()
    out = nc.dram_tensor("out", (S, D), F32, kind="ExternalOutput").ap()

    with tile.TileContext(nc) as tc, ExitStack() as ctx:
        wpool = ctx.enter_context(tc.tile_pool(name="w", bufs=1))
        cpool = ctx.enter_context(tc.tile_pool(name="const", bufs=1))
        xpool = ctx.enter_context(tc.tile_pool(name="x", bufs=bufs))
        tpool = ctx.enter_context(tc.tile_pool(name="t", bufs=bufs))
        spool = ctx.enter_context(tc.tile_pool(name="sm", bufs=bufs))
        prpool = ctx.enter_context(tc.tile_pool(name="pr", bufs=bufs))
        pvpool = ctx.enter_context(tc.tile_pool(name="pv", bufs=1))
        ypool = ctx.enter_context(
            tc.tile_pool(name="y", bufs=cfg.get("lag", 2) + 1))
        ytpool = ctx.enter_context(tc.tile_pool(name="yt", bufs=1))
        opool = ctx.enter_context(tc.tile_pool(name="o", bufs=1))
        pp_t = ctx.enter_context(tc.tile_pool(name="ps_t", bufs=4, space="PSUM"))
        pp_tr = ctx.enter_context(tc.tile_pool(name="ps_tr", bufs=2, space="PSUM"))
        pp_o = ctx.enter_context(tc.tile_pool(name="ps_o", bufs=2, space="PSUM"))

        # chunked weight loads as separate tiles: the first t-matmul only
        # needs wq chunk 0, so the pipeline starts ~6us in instead of
        # after the full 38us DMA
        # preload the first two x tiles before any weight DMA so the SP
        # queue serves them first and the PE can start at ~6us
        ident = cpool.tile([128, 128], F16, tag="id")
        make_identity(nc, ident[:])
        screbufs = []
        for kk in range(2):
            sc = cpool.tile([128, H * W], F32, tag=f"sc{kk}", name=f"screct{kk}")
            nc.gpsimd.memset(sc[:], NEG)
            screbufs.append(sc)

        xts = {}
        for n0 in range(2):
            xt0 = xpool.tile([128, KB, TILE], F32R, tag="xt", name=f"xt{n0}")
            nc.sync.dma_start(
                xt0[:], xT[:, n0 * TILE:(n0 + 1) * TILE].rearrange(
                    "(kb p) s -> p kb s", p=128))
            xts[n0] = xt0

        # each dma_start occupies the issuing engine's DMA queue for the
        # whole transfer -> spread the weight chunks over all idle queues
        wq_cs = []
        dma_engs = [nc.gpsimd, nc.scalar, nc.sync,
                    nc.sync, nc.scalar, nc.sync]
        for oc in range(6):
            wq_c = wpool.tile([128, KB, 512], F32R, tag=f"wq{oc}",
                              name=f"wq{oc}")
            dma_engs[oc].dma_start(
                wq_c[:], _rearr_kb(wq)[:, :, oc * 512:(oc + 1) * 512])
            wq_cs.append(wq_c)
        wo_t = wpool.tile([128, KB, D], BF16, tag="wo")
        nc.sync.dma_start(wo_t[:], _rearr_kb(wo))

        yps = {}
        vts = {}

        def frontA(n):
            sl = slice(n * TILE, (n + 1) * TILE)

            if n in xts:
                xt = xts.pop(n)
            else:
                xt = xpool.tile([128, KB, TILE], F32R, tag="xt")
                nc.sync.dma_start(
                    xt[:], xT[:, sl].rearrange("(kb p) s -> p kb s", p=128))

            # ---- step 1: t = x @ w_qkv.T into 6 psum chunks -------------
            ps_t = [None] * 6
            for oc in (1, 2, 0, 3, 4, 5):   # weight-arrival order at startup
                ps = pp_t.tile([128, 512], F32, tag="t", name=f"pst{n}_{oc}")
                for kb in range(KB):
                    nc.tensor.matmul(
                        ps[:], xt[:, kb, :], wq_cs[oc][:, kb, :],
                        start=(kb == 0), stop=(kb == KB - 1))
                ps_t[oc] = ps

            # ---- drains -------------------------------------------------
            q_sb = tpool.tile([128, D], F32, tag="q")
            k_sb = tpool.tile([128, D], F32, tag="k")
            vt = tpool.tile([128, D], F16, tag="vt")
            # q-hi and k-lo drain first: diagonals 8..13 read only
            # those halves, so score work starts one PE-chunk earlier
            nc.scalar.copy(q_sb[:, 512:1024], ps_t[1][:])
            nc.scalar.copy(k_sb[:, 0:512], ps_t[2][:])
            nc.scalar.copy(q_sb[:, 0:512], ps_t[0][:])
            nc.scalar.copy(k_sb[:, 512:1024], ps_t[3][:])
            # vt[:, 16*d + j] = v[:, 64*j + d]
            for h in range(2):
                src = ps_t[4 + h][:].rearrange("p (j d) -> p j d", d=DH)
                dst = vt[:].rearrange("p (d j) -> p d j", j=H)[
                    :, :, h * 8:(h + 1) * 8].rearrange("p d j -> p j d")
                nc.scalar.copy(dst, src)

            # ---- scores: one multiply + reduce per diagonal (i-j = o) ---
            screct = screbufs[n % 2]
            q3 = q_sb[:].rearrange("p (i d) -> p i d", d=DH)
            k3 = k_sb[:].rearrange("p (j d) -> p j d", d=DH)
            # per diagonal: Pool f32 mul; for l1_diags an extra Pool fold
            # (add d-halves) so the DVE-only segmented reduce reads half.
            # Real-HW note: Q7 gpsimd runs TensorTensor at ~2ns/elem
            # (Multiply impl efficiency 0.42, A/B-measured on HW), so Pool
            # only carries the muls + a few folds; everything else is DVE.
            prodA = prpool.tile([128, H * DH], F32, tag="prodA", name="prodA")
            prodBs = [prpool.tile([128, (H - 4 - b_) * DH], F32,
                                  tag=f"prodB{b_}", name=f"prodB{b_}")
                      for b_ in range(2)]
            phs = [prpool.tile([128, (H - b_) * (DH // 2)], F32,
                               tag=f"ph{b_}", name=f"ph{b_}")
                   for b_ in range(2)]
            for o in [8, 9, 10, 11, 12, 13, 14, 15, 0, 1, 2, 3, 4, 5, 6, 7]:
                cnt = H - o
                prod = prodA if o in l1_diags else prodBs[o % 2]
                pr3 = prod[:, 0:cnt * DH].rearrange("p (j d) -> p j d", d=DH)
                meng = nc.gpsimd if o in pool_mul else nc.vector
                meng.tensor_mul(pr3, q3[:, o:H, :], k3[:, 0:cnt, :])
                red_in = pr3
                if o in l1_diags:
                    ph = phs[o % 2]
                    ph3 = ph[:, 0:cnt * (DH // 2)].rearrange(
                        "p (j d) -> p j d", d=DH // 2)
                    meng.tensor_add(
                        ph3, pr3[:, :, 0:DH // 2], pr3[:, :, DH // 2:])
                    red_in = ph3
                nc.vector.reduce_sum(
                    screct[:, W * o:W * o + (W + 1) * (cnt - 1) + 1:W + 1],
                    red_in, axis=mybir.AxisListType.X)

            vts[n] = vt

        def frontB(n):
            # ---- softmax over j -----------------------------------------
            # cols 16/17 of the W=18 rect are never consumed downstream
            # (p@v reads j<16; in-range pads already exp to 0), so all
            # softmax ops process only [0:16) -- row stride stays W
            screct = screbufs[n % 2]
            vt = vts.pop(n)
            sc3 = screct[:].rearrange("p (i j) -> p i j", j=W)[:, :, 0:H]
            mrow = spool.tile([128, H], F32, tag="m")
            nc.vector.reduce_max(mrow[:], sc3, axis=mybir.AxisListType.X)
            mb = mrow[:].rearrange("p (i one) -> p i one", one=1).broadcast_to(
                (128, H, H))
            shifted = spool.tile([128, H * W], F16, tag="sh")
            sh3 = shifted[:].rearrange("p (i j) -> p i j", j=W)[:, :, 0:H]
            (nc.gpsimd if sub_eng == "pool" else nc.vector).tensor_sub(
                sh3, sc3, mb)
            praw = spool.tile([128, H * W], F16, tag="e")
            pr3v = praw[:].rearrange("p (i j) -> p i j", j=W)[:, :, 0:H]
            nc.scalar.activation(pr3v, sh3, mybir.ActivationFunctionType.Exp)
            zrow = spool.tile([128, H], F32, tag="z")
            nc.vector.reduce_sum(zrow[:], pr3v, axis=mybir.AxisListType.X)
            rrow = spool.tile([128, H], F32, tag="r")
            nc.vector.reciprocal(rrow[:], zrow[:])
            prect = spool.tile([128, H * W], F16, tag="p")
            rb = rrow[:].rearrange("p (i one) -> p i one", one=1).broadcast_to(
                (128, H, H))
            # Act replicates 1/Z across the row so the DVE normalize-mul
            # sees packed fp16 operands (2x mode) instead of a 0-stride
            # broadcast (1x)
            rbr = spool.tile([128, H * W], F16, tag="rbr")
            rbr3 = rbr[:].rearrange("p (i j) -> p i j", j=W)[:, :, 0:H]
            nc.scalar.copy(rbr3, rb)
            (nc.gpsimd if norm_eng == "pool" else nc.vector).tensor_mul(
                prect[:].rearrange("p (i j) -> p i j", j=W)[:, :, 0:H],
                pr3v, rbr3)

            # ---- p @ v: batched pow2-class muls + fold-halves tree ------
            p3 = prect[:].rearrange("p (i j) -> p i j", j=W)
            vt3 = vt[:].rearrange("p (d j) -> p d j", j=H)
            yp = ypool.tile([128, D], F16, tag="yp")

            # class (8,4,12): i=8..11 true nj<=12, so a 12-wide product
            # with an 8+4 subtree merge beats padding to 16 (saves
            # ~1.3us/tile DVE; slots reused from the 16-wide class's tags)
            pc12 = pvpool.tile([128, 4 * DH * 16], F16, tag="pc4_16",
                               name="pc12")
            d12 = pc12[:, 0:4 * DH * 12].rearrange(
                "p (i d j) -> p i d j", i=4, d=DH)
            pin = p3[:, 8:12, 0:12].rearrange(
                "p i (one j) -> p i one j", one=1).broadcast_to(
                (128, 4, DH, 12))
            vin = vt3[:, :, 0:12].rearrange(
                "p (one d) j -> p one d j", one=1).broadcast_to(
                (128, 4, DH, 12))
            nc.vector.tensor_mul(d12, pin, vin)
            fA1 = pvpool.tile([128, 4 * DH * 4], F16, tag="f4_4",
                              name="fA1")
            a1 = fA1[:].rearrange("p (i d j) -> p i d j", i=4, d=DH)
            nc.vector.tensor_add(a1, d12[:, :, :, 0:4], d12[:, :, :, 4:8])
            fB1 = pvpool.tile([128, 4 * DH * 8], F16, tag="f4_8",
                              name="fB1")
            b1 = fB1[:, 0:4 * DH * 2].rearrange(
                "p (i d j) -> p i d j", i=4, d=DH)
            nc.vector.tensor_add(b1, d12[:, :, :, 8:10], d12[:, :, :, 10:12])
            fA2 = pvpool.tile([128, 4 * DH * 2], F16, tag="f4_2",
                              name="fA2")
            a2 = fA2[:].rearrange("p (i d j) -> p i d j", i=4, d=DH)
            nc.vector.tensor_add(a2, a1[:, :, :, 0:2], a1[:, :, :, 2:4])
            fM = pvpool.tile([128, 4 * DH * 4], F16, tag="f4_4", name="fM")
            m2 = fM[:, 0:4 * DH * 2].rearrange(
                "p (i d j) -> p i d j", i=4, d=DH)
            nc.vector.tensor_add(m2, a2, b1)
            ypd = yp[:, 8 * DH:12 * DH].rearrange(
                "p (i d one) -> p i d one", i=4, one=1)
            nc.vector.tensor_add(ypd, m2[:, :, :, 0:1], m2[:, :, :, 1:2])

            for (i0, ic, nj) in PV_CLASSES:
                # products pp[p, i, d, j] = prect[p, i0+i, j] * vt[p, d, j]
                if nj == 1:
                    dst = yp[:, i0 * DH:(i0 + ic) * DH].rearrange(
                        "p (i d one) -> p i d one", i=ic, one=1)
                else:
                    pc = pvpool.tile([128, ic * DH * nj], F16,
                                     tag=f"pc{ic}_{nj}", name=f"pc{ic}_{nj}")
                    dst = pc[:].rearrange(
                        "p (i d j) -> p i d j", i=ic, d=DH)
                pin = p3[:, i0:i0 + ic, 0:nj].rearrange(
                    "p i (one j) -> p i one j", one=1).broadcast_to(
                    (128, ic, DH, nj))
                vin = vt3[:, :, 0:nj].rearrange(
                    "p (one d) j -> p one d j", one=1).broadcast_to(
                    (128, ic, DH, nj))
                nc.vector.tensor_mul(dst, pin, vin)
                # fold-halves: m -> m/2 -> ... -> 1, last level into yp
                m = nj
                cur = dst
                while m > 1:
                    m //= 2
                    if m == 1:
                        nxt = yp[:, i0 * DH:(i0 + ic) * DH].rearrange(
                            "p (i d one) -> p i d one", i=ic, one=1)
                    else:
                        fbuf = pvpool.tile(
                            [128, ic * DH * m], F16, tag=f"f{ic}_{m}",
                            name=f"f{ic}_{m}")
                        nxt = fbuf[:].rearrange(
                            "p (i d j) -> p i d j", i=ic, d=DH)
                    feng = (nc.gpsimd
                            if (nj, m) in cfg.get("pool_pv_folds", ())
                            else nc.vector)
                    feng.tensor_add(
                        nxt, cur[:, :, :, 0:m], cur[:, :, :, m:2 * m])
                    cur = nxt

            yps[n] = yp

        def back(n):
            sl = slice(n * TILE, (n + 1) * TILE)
            yp = yps.pop(n)
            # ---- transpose y', then out = y' @ w_o.T --------------------
            # kb order follows yp-slice completion (big head classes
            # finish first), so the PE transposes/wo overlap the p@v
            # tail instead of gating on the last-written head slice
            KB_ORDER = [4, 5, 6, 7, 2, 3, 1, 0]
            ypT = ytpool.tile([128, KB, TILE], BF16, tag="ypT")
            ps_tr = [pp_tr.tile([128, 512], F16, tag="tr", name=f"tr{n}_{h}")
                     for h in range(2)]
            for kb in KB_ORDER:
                nc.tensor.transpose(
                    ps_tr[kb // 4][:, (kb % 4) * 128:(kb % 4 + 1) * 128],
                    yp[:, kb * 128:(kb + 1) * 128], ident[:])
            ypTf = ypT[:].rearrange("p kb s -> p (kb s)")
            nc.scalar.copy(ypTf[:, 512:1024], ps_tr[1][:])
            nc.scalar.copy(ypTf[:, 256:512], ps_tr[0][:, 256:512])
            nc.scalar.copy(ypTf[:, 128:256], ps_tr[0][:, 128:256])
            nc.scalar.copy(ypTf[:, 0:128], ps_tr[0][:, 0:128])

            osb = opool.tile([128, D], F32, tag="osb")
            for oc in range(2):
                ps_o = pp_o.tile([128, 512], F32, tag="o", name=f"pso{n}_{oc}")
                for ki, kb in enumerate(KB_ORDER):
                    nc.tensor.matmul(
                        ps_o[:], ypT[:, kb, :], wo_t[:, kb, oc * 512:(oc + 1) * 512],
                        start=(ki == 0), stop=(ki == KB - 1))
                nc.scalar.copy(osb[:, oc * 512:(oc + 1) * 512], ps_o[:])
                nc.sync.dma_start(out[sl, oc * 512:(oc + 1) * 512],
                                  osb[:, oc * 512:(oc + 1) * 512])

        # software pipeline, 3 stages:
        #   frontB(n-1): softmax + p@v of the PREVIOUS tile is emitted
        #     before frontA(n)'s score reduces, so on the in-order DVE
        #     queue p@v(n-1) (inputs ready) runs while Pool chews tile
        #     n's score muls -- instead of Pool idling through p@v and
        #     DVE then stalling on Pool's first muls of the next tile.
        #   frontA(n): x DMA, t-matmuls, drains, score muls+reduces.
        #   back(n-2): y'-transpose + out-proj emitted 2 tiles late so
        #     the PE queue always has t(n) ahead of the wait-for-p@v
        #     transpose (keeps PE busy, HAM stays warm).
        lag = cfg.get("lag", 2)
        if cfg.get("reorder", True):
            for it in range(N_TILES + lag + 1):
                if 1 <= it <= N_TILES:
                    frontB(it - 1)
                if it < N_TILES:
                    frontA(it)
                if it >= lag + 1:
                    back(it - lag - 1)
        else:
            for it in range(N_TILES + lag):
                if it < N_TILES:
                    frontA(it)
                    frontB(it)
                if it >= lag:
                    back(it - lag)

    return nc


# --------------------------------------------------------------------------
# host side
# --------------------------------------------------------------------------
_CACHE = {}


def _get_nc():
    if "nc" not in _CACHE:
        nc = bass.Bass("TRN2", target_bir_lowering=False, debug=False,
                       num_devices=N_CORES)
        build(nc, CFG)
        split_excess_waits(nc)
        _CACHE["nc"] = nc
    return _CACHE["nc"]


def _host_inputs(x, w_qkv, w_o):
    import ml_dtypes
    wq = np.ascontiguousarray(w_qkv.T).astype(np.float32).copy()
    wq[:, :D] *= np.float32(1.0 / math.sqrt(D))
    wo = np.ascontiguousarray(w_o.T).astype(ml_dtypes.bfloat16)
    shared = {"wq": wq, "wo": wo}
    in_maps = []
    for b in range(B):
        xT = np.ascontiguousarray(x[b].T)
        in_maps.append({"xT": xT, **shared})
    return in_maps


def kernel(x, w_qkv, w_o, n_heads=H, **_unused):
    from concourse import bass_utils

    x = np.asarray(x, dtype=np.float32)
    w_qkv = np.asarray(w_qkv, dtype=np.float32)
    w_o = np.asarray(w_o, dtype=np.float32)
    assert int(n_heads) == H
    assert x.shape == (B, S, D), x.shape

    nc = _get_nc()
    in_maps = _host_inputs(x, w_qkv, w_o)
    res = bass_utils.run_bass_kernel_spmd(
        nc, in_maps, core_ids=list(range(N_CORES)))
    out = np.stack([res.results[b]["out"] for b in range(B)])
    return out.astype(np.float32)

